# revision 13
# baseline (speedup 1.0000x reference)
"""Fake-quantized linear layer (int8 symmetric fake-quant) on 8 TRN2 NeuronCores.

Reference computation:
    sx = max(|x|)/127            (per-tensor, scalar)
    sw[o] = max(|w[o,:]|)/127    (per-output-channel)
    qx = round(clip(x/sx, -127, 127));  qw = round(clip(w/sw, -127, 127))
    y = (qx*sx) @ (qw*sw).T + bias
      = (qx @ qw.T) * (sx*sw[o]) + bias    -- exact integer arithmetic

Device strategy (pure data-parallel over tokens, no collectives):
  - 16384 tokens sharded 2048/core; quantized weight + bias replicated.
  - Weights are quantized on the host (the standard int8-inference split:
    weights quantized offline, activations quantized on the fly); the
    integer-valued qw ships as bf16 (ints <= 127 are exact in bf16), already
    transposed to [Din, Dout] so the contraction dim lands on partitions.
  - Activations are quantized on device: x*(1/sx) then the fp32
    magic-constant round trick ((v + 12582912.0) - 12582912.0 ==
    round-half-even(v) for |v| < 2^22), emitted as bf16.
  - Matmul runs on the TensorEngine in bf16 with fp32 PSUM accumulation,
    then the output is scaled by sx*sw[o] and bias is added (VectorEngine),
    and written out f32.
  - SBUF: the quantized x shard is kept resident one 1024-token half at a
    time ([128, 32, 1024] bf16 = 64 KiB/partition); the other half stages
    through DRAM. qw tiles ([128, 32, 512] bf16) are DMA'd directly (no
    on-device quant work), double-buffered; qw is read twice (once per
    token half).

Execution/timing notes (axon-tunneled PJRT):
  - Any client-side await (block_until_ready, device_put of even a few
    bytes) costs a fixed ~100 ms round trip through the tunnel. Back-to-back
    submitted executions pipeline on the terminal with ~1 ms marginal cost
    each (measured: 16 trivial execs complete in ~115 ms total; a chain of 8
    dependent execs in ~106 ms). So single-shot wall time is tunnel-latency
    dominated and says nothing about kernel speed.
  - bench() therefore measures sustained HW execution time: submit K
    donation-chained executions (exec i+1 donates exec i's output buffer, so
    they run strictly back-to-back on device), block once, divide by K.
  - All 8 cores run in a single shard_map dispatch.
"""

import os
import time

import numpy as np

import concourse.bacc as bacc
import concourse.mybir as mybir
import concourse.tile as tile
from concourse.bass_utils import run_bass_kernel_spmd  # noqa: F401 (debug path)

N_CORES = 8
P = 128
DIN = 4096
DOUT = 4096
T = 2048           # tokens per core
TH = 1024          # token half
KO = DIN // P      # 32 k-subtiles
NE = DOUT // 512   # 8 dout-eighths
MH = TH // P       # 8 m-subtiles per half
C_MAGIC = 12582912.0  # 2^23 + 2^22: fp32 round-to-nearest-even magic


def build(DIN=DIN, DOUT=DOUT, T=T, num_devices=N_CORES,
          psum_bufs=3, opool_bufs=3, xstage_bufs=2, variant="n512_staged",
          reps=1):
    """Build the kernel NEFF. reps>1 emits the full computation that many
    times back-to-back inside one NEFF (same inputs, same output buffer,
    each rep fully rewrites y) — used only for steady-state benchmarking;
    the result is identical to reps=1."""
    TH = T // 2
    KO = DIN // P
    NE = DOUT // 512
    MH = TH // P
    nc = bacc.Bacc("TRN2", target_bir_lowering=False, debug=False,
                   num_devices=num_devices)
    f32 = mybir.dt.float32
    bf16 = mybir.dt.bfloat16

    xT = nc.dram_tensor("xT", [DIN, T], f32, kind="ExternalInput")
    wqT = nc.dram_tensor("wqT", [DIN, DOUT], bf16, kind="ExternalInput")
    sc = nc.dram_tensor("sc", [DOUT], f32, kind="ExternalInput")     # sx*sw
    bi = nc.dram_tensor("bi", [DOUT], f32, kind="ExternalInput")     # bias
    rx = nc.dram_tensor("rx", [P, 1], f32, kind="ExternalInput")     # 1/sx
    y = nc.dram_tensor("y", [T, DOUT], f32, kind="ExternalOutput")

    if variant == "n256_resident":
        return _build_n256(nc, xT, wqT, sc, bi, rx, y,
                           DIN, DOUT, T, psum_bufs, opool_bufs)
    if variant == "dualres":
        return _build_dualres(nc, xT, wqT, sc, bi, rx, y,
                              DIN, DOUT, T, psum_bufs, opool_bufs, reps)

    with tile.TileContext(nc) as tc:
        with tc.tile_pool(name="xres", bufs=1) as xres, \
             tc.tile_pool(name="wq", bufs=2) as wqp, \
             tc.tile_pool(name="xstage", bufs=xstage_bufs) as xstage, \
             tc.tile_pool(name="xfstage", bufs=xstage_bufs) as xfstage, \
             tc.tile_pool(name="xq1stage", bufs=2) as xq1stage, \
             tc.tile_pool(name="rowbc", bufs=2) as rowbc, \
             tc.tile_pool(name="opool", bufs=opool_bufs) as opool, \
             tc.tile_pool(name="scal", bufs=1) as scal, \
             tc.tile_pool(name="dram", bufs=1, space="DRAM") as dram, \
             tc.tile_pool(name="psum", bufs=psum_bufs, space="PSUM") as psum:

            rxt = scal.tile([P, 1], f32)
            nc.sync.dma_start(rxt[:], rx.ap())

            def quant_x(c, ko, xq0, xq1_dram):
                xt = xstage.tile([P, TH], f32)
                nc.sync.dma_start(
                    xt[:], xT.ap()[ko * P:(ko + 1) * P, c * TH:(c + 1) * TH])
                xf = xfstage.tile([P, TH], f32)
                # xf = xt*(1/sx) + C  (scalar engine)
                nc.scalar.activation(xf[:], xt[:],
                                     mybir.ActivationFunctionType.Copy,
                                     bias=C_MAGIC, scale=rxt[:])
                # subtract C -> round-half-even(xt/sx), emit bf16
                if c == 0:
                    nc.vector.tensor_scalar(xq0[:, ko, :], xf[:], -C_MAGIC,
                                            None, mybir.AluOpType.add)
                else:
                    xq1 = xq1stage.tile([P, TH], bf16)
                    nc.vector.tensor_scalar(xq1[:], xf[:], -C_MAGIC,
                                            None, mybir.AluOpType.add)
                    nc.sync.dma_start(xq1_dram[ko], xq1[:])

            for _rep in range(reps):
                xq1_dram = dram.tile([KO, P, TH], bf16)

                # ---- Phase X: quantize x; half 0 -> resident SBUF,
                # half 1 -> DRAM staging (reloaded at half switch).
                xq0 = xres.tile([P, KO, TH], bf16, tag="xres")
                for ko in range(KO):
                    quant_x(0, ko, xq0, xq1_dram)
                for ko in range(KO):
                    quant_x(1, ko, xq0, xq1_dram)

                # ---- Main loop: token halves x dout eighths
                for h in range(2):
                    if h == 0:
                        xq = xq0
                    else:
                        xq = xres.tile([P, KO, TH], bf16, tag="xres")
                        nc.sync.dma_start(
                            xq[:], xq1_dram[:].rearrange("k p t -> p k t"))
                    for ne in range(NE):
                        ds = slice(ne * 512, (ne + 1) * 512)
                        scb = rowbc.tile([P, 512], f32, tag="scb")
                        nc.sync.dma_start(scb[:],
                                          sc.ap()[ds].partition_broadcast(P))
                        bib = rowbc.tile([P, 512], f32, tag="bib")
                        nc.sync.dma_start(bib[:],
                                          bi.ap()[ds].partition_broadcast(P))

                        wq = wqp.tile([P, KO, 512], bf16)
                        for ko in range(KO):
                            nc.sync.dma_start(
                                wq[:, ko, :], wqT.ap()[ko * P:(ko + 1) * P, ds])

                        for m in range(MH):
                            ps = psum.tile([P, 512], f32)
                            for k in range(KO):
                                nc.tensor.matmul(
                                    ps[:], xq[:, k, m * P:(m + 1) * P],
                                    wq[:, k, :],
                                    start=(k == 0), stop=(k == KO - 1))
                            ot = opool.tile([P, 512], f32)
                            nc.vector.tensor_tensor(ot[:], ps[:], scb[:],
                                                    mybir.AluOpType.mult)
                            nc.vector.tensor_tensor(ot[:], ot[:], bib[:],
                                                    mybir.AluOpType.add)
                            row = h * TH + m * P
                            nc.sync.dma_start(y.ap()[row:row + P, ds], ot[:])

    nc.compile()
    return nc


def _build_dualres(nc, xT, wqT, sc, bi, rx, y, DIN, DOUT, T,
                   psum_bufs, opool_bufs, reps):
    """Variant: full quantized-x residency ([128, 32, 2048] bf16 =
    128 KiB/partition) at N=512, no DRAM staging. Weight blocks are
    double-buffered at half-K granularity ([128, 16, 512] bf16 = 16 KiB,
    3 bufs) so everything fits in SBUF. x is quantized in token-major
    order so the first matmul group is gated on only 1/4 of the x DMA."""
    f32 = mybir.dt.float32
    bf16 = mybir.dt.bfloat16
    KO = DIN // P
    KO2 = KO // 2
    NE = DOUT // 512
    MT = T // P
    TQ = 512
    NQ = T // TQ

    with tile.TileContext(nc) as tc:
        with tc.tile_pool(name="xres", bufs=1) as xres, \
             tc.tile_pool(name="wq", bufs=3) as wqp, \
             tc.tile_pool(name="xstage", bufs=2) as xstage, \
             tc.tile_pool(name="xfstage", bufs=2) as xfstage, \
             tc.tile_pool(name="rowbc", bufs=2) as rowbc, \
             tc.tile_pool(name="opool", bufs=opool_bufs) as opool, \
             tc.tile_pool(name="scal", bufs=1) as scal, \
             tc.tile_pool(name="psum", bufs=psum_bufs, space="PSUM") as psum:

            rxt = scal.tile([P, 1], f32)
            nc.sync.dma_start(rxt[:], rx.ap())

            for _rep in range(reps):
                xq = xres.tile([P, KO, T], bf16, tag="xres")
                for c in range(NQ):
                    for ko in range(KO):
                        xt = xstage.tile([P, TQ], f32)
                        nc.sync.dma_start(
                            xt[:],
                            xT.ap()[ko * P:(ko + 1) * P,
                                    c * TQ:(c + 1) * TQ])
                        xf = xfstage.tile([P, TQ], f32)
                        nc.scalar.activation(
                            xf[:], xt[:],
                            mybir.ActivationFunctionType.Copy,
                            bias=C_MAGIC, scale=rxt[:])
                        nc.vector.tensor_scalar(
                            xq[:, ko, c * TQ:(c + 1) * TQ], xf[:], -C_MAGIC,
                            None, mybir.AluOpType.add)

                for ne in range(NE):
                    ds = slice(ne * 512, (ne + 1) * 512)
                    scb = rowbc.tile([P, 512], f32, tag="scb")
                    nc.sync.dma_start(scb[:],
                                      sc.ap()[ds].partition_broadcast(P))
                    bib = rowbc.tile([P, 512], f32, tag="bib")
                    nc.sync.dma_start(bib[:],
                                      bi.ap()[ds].partition_broadcast(P))

                    wk = []
                    for kh in range(2):
                        wt = wqp.tile([P, KO2, 512], bf16)
                        for k in range(KO2):
                            ko = kh * KO2 + k
                            nc.sync.dma_start(
                                wt[:, k, :],
                                wqT.ap()[ko * P:(ko + 1) * P, ds])
                        wk.append(wt)

                    for m in range(MT):
                        ps = psum.tile([P, 512], f32)
                        for kh in range(2):
                            for k in range(KO2):
                                ko = kh * KO2 + k
                                nc.tensor.matmul(
                                    ps[:], xq[:, ko, m * P:(m + 1) * P],
                                    wk[kh][:, k, :],
                                    start=(ko == 0), stop=(ko == KO - 1))
                        ot = opool.tile([P, 512], f32)
                        nc.vector.tensor_tensor(ot[:], ps[:], scb[:],
                                                mybir.AluOpType.mult)
                        nc.vector.tensor_tensor(ot[:], ot[:], bib[:],
                                                mybir.AluOpType.add)
                        nc.sync.dma_start(y.ap()[m * P:(m + 1) * P, ds], ot[:])

    nc.compile()
    return nc


def _build_n256(nc, xT, wqT, sc, bi, rx, y, DIN, DOUT, T,
                psum_bufs, opool_bufs):
    """Variant: full quantized-x residency ([128, 32, 2048] bf16 =
    128 KiB/partition), no DRAM staging and no half-switch reload bubble;
    weight blocks narrowed to 256 outs so double-buffered qw still fits."""
    f32 = mybir.dt.float32
    bf16 = mybir.dt.bfloat16
    TH = T // 2
    KO = DIN // P
    NB = 256
    NEB = DOUT // NB
    MT = T // P

    with tile.TileContext(nc) as tc:
        with tc.tile_pool(name="xres", bufs=1) as xres, \
             tc.tile_pool(name="wq", bufs=2) as wqp, \
             tc.tile_pool(name="xstage", bufs=2) as xstage, \
             tc.tile_pool(name="xfstage", bufs=2) as xfstage, \
             tc.tile_pool(name="rowbc", bufs=2) as rowbc, \
             tc.tile_pool(name="opool", bufs=opool_bufs) as opool, \
             tc.tile_pool(name="scal", bufs=1) as scal, \
             tc.tile_pool(name="psum", bufs=psum_bufs, space="PSUM") as psum:

            rxt = scal.tile([P, 1], f32)
            nc.sync.dma_start(rxt[:], rx.ap())

            xq = xres.tile([P, KO, T], bf16)
            for c in range(2):
                for ko in range(KO):
                    xt = xstage.tile([P, TH], f32)
                    nc.sync.dma_start(
                        xt[:],
                        xT.ap()[ko * P:(ko + 1) * P, c * TH:(c + 1) * TH])
                    xf = xfstage.tile([P, TH], f32)
                    nc.scalar.activation(xf[:], xt[:],
                                         mybir.ActivationFunctionType.Copy,
                                         bias=C_MAGIC, scale=rxt[:])
                    nc.vector.tensor_scalar(
                        xq[:, ko, c * TH:(c + 1) * TH], xf[:], -C_MAGIC,
                        None, mybir.AluOpType.add)

            for ne in range(NEB):
                ds = slice(ne * NB, (ne + 1) * NB)
                scb = rowbc.tile([P, NB], f32, tag="scb")
                nc.sync.dma_start(scb[:], sc.ap()[ds].partition_broadcast(P))
                bib = rowbc.tile([P, NB], f32, tag="bib")
                nc.sync.dma_start(bib[:], bi.ap()[ds].partition_broadcast(P))

                wq = wqp.tile([P, KO, NB], bf16)
                for ko in range(KO):
                    nc.sync.dma_start(
                        wq[:, ko, :], wqT.ap()[ko * P:(ko + 1) * P, ds])

                for m in range(MT):
                    ps = psum.tile([P, NB], f32)
                    for k in range(KO):
                        nc.tensor.matmul(
                            ps[:], xq[:, k, m * P:(m + 1) * P], wq[:, k, :],
                            start=(k == 0), stop=(k == KO - 1))
                    ot = opool.tile([P, NB], f32)
                    nc.vector.tensor_tensor(ot[:], ps[:], scb[:],
                                            mybir.AluOpType.mult)
                    nc.vector.tensor_tensor(ot[:], ot[:], bib[:],
                                            mybir.AluOpType.add)
                    nc.sync.dma_start(y.ap()[m * P:(m + 1) * P, ds], ot[:])

    nc.compile()
    return nc


_NC_CACHE = {}


def _get_nc(reps=1):
    key = ("nc", reps)
    if key not in _NC_CACHE:
        variant = os.environ.get("KERNEL_VARIANT", "n512_staged")
        _NC_CACHE[key] = build(variant=variant, reps=reps)
    return _NC_CACHE[key]


def _get_runner(dev_lo, dev_hi, reps=1):
    """Compiled shard_map runner for jax devices [dev_lo, dev_hi).

    Mirrors concourse.bass2jax.run_bass_via_pjrt's multi-core path, but
    caches the jitted executable and exposes helpers for device-resident
    pipelined benching (donation-chained repeat executions).
    """
    key = (dev_lo, dev_hi, reps)
    if key in _NC_CACHE:
        return _NC_CACHE[key]

    import jax
    import jax.numpy as jnp
    from jax.sharding import Mesh, PartitionSpec
    from jax.experimental.shard_map import shard_map
    from concourse import bass2jax, mybir as _mybir

    nc = _get_nc(reps)
    bass2jax.install_neuronx_cc_hook()

    partition_name = (nc.partition_id_tensor.name
                      if nc.partition_id_tensor else None)
    in_names, out_names, out_avals, zero_outs = [], [], [], []
    for alloc in nc.m.functions[0].allocations:
        if not isinstance(alloc, _mybir.MemoryLocationSet):
            continue
        name = alloc.memorylocations[0].name
        if alloc.kind == "ExternalInput":
            if name != partition_name:
                in_names.append(name)
        elif alloc.kind == "ExternalOutput":
            shape = tuple(alloc.tensor_shape)
            dtype = _mybir.dt.np(alloc.dtype)
            out_names.append(name)
            out_avals.append(jax.core.ShapedArray(shape, dtype))
            zero_outs.append(np.zeros(shape, dtype))
    n_params = len(in_names)
    n_outs = len(out_avals)
    all_names = in_names + out_names
    if partition_name is not None:
        all_names = all_names + [partition_name]
    donate = tuple(range(n_params, n_params + n_outs))
    n_cores = dev_hi - dev_lo

    def _body(*args):
        operands = list(args)
        if partition_name is not None:
            operands.append(bass2jax.partition_id_tensor())
        outs = bass2jax._bass_exec_p.bind(
            *operands,
            out_avals=tuple(out_avals),
            in_names=tuple(all_names),
            out_names=tuple(out_names),
            lowering_input_output_aliases=(),
            sim_require_finite=True,
            sim_require_nnan=True,
            nc=nc,
        )
        return tuple(outs)

    devices = jax.devices()[dev_lo:dev_hi]
    mesh = Mesh(np.asarray(devices), ("core",))
    in_specs = (PartitionSpec("core"),) * (n_params + n_outs)
    out_specs = (PartitionSpec("core"),) * n_outs
    jitted = jax.jit(
        shard_map(_body, mesh=mesh, in_specs=in_specs, out_specs=out_specs,
                  check_rep=False),
        donate_argnums=donate, keep_unused=True)

    sharding = jax.sharding.NamedSharding(mesh, PartitionSpec("core"))

    def concat_inputs(in_maps):
        assert len(in_maps) == n_cores
        return [
            np.concatenate([np.asarray(m[name]) for m in in_maps], axis=0)
            for name in in_names
        ]

    # Donated output buffers are created on device (jnp.zeros under jit) --
    # the kernel writes every output element, so contents don't matter, but
    # this avoids shipping 100s of MB of host zeros through the tunnel.
    zshapes = [((n_cores * z.shape[0],) + z.shape[1:], z.dtype)
               for z in zero_outs]
    dev_zeros = jax.jit(
        lambda: tuple(jnp.zeros(s, d) for s, d in zshapes),
        out_shardings=tuple(sharding for _ in zshapes))

    def run(in_maps):
        dev_in = [jax.device_put(a, sharding) for a in concat_inputs(in_maps)]
        return jitted(*dev_in, *dev_zeros())

    run.jitted = jitted
    run.concat_inputs = concat_inputs
    run.dev_zeros = dev_zeros
    run.sharding = sharding

    def unpack(out_arrs):
        return [
            {name: np.asarray(out_arrs[i]).reshape(
                n_cores, *out_avals[i].shape)[c]
             for i, name in enumerate(out_names)}
            for c in range(n_cores)
        ]

    _NC_CACHE[key] = (run, unpack)
    return _NC_CACHE[key]


def _runners(neff_reps=1):
    group = int(os.environ.get("KERNEL_CORE_GROUP", "8"))
    return group, [_get_runner(g0, g0 + group, reps=neff_reps)
                   for g0 in range(0, N_CORES, group)]


def bench(in_maps, reps=5, pipeline=64, neff_reps=1):
    """Measure sustained per-execution HW time.

    Inputs are device_put once (outside any timer). Each rep submits
    `pipeline` donation-chained executions (exec i+1 consumes exec i's
    output buffer, so they run strictly back-to-back on the device) and
    blocks once; rep time = total / (pipeline * neff_reps). With
    neff_reps>1 each execution's NEFF contains that many back-to-back
    repetitions of the full kernel computation, so per-kernel runtime
    dispatch overhead is amortized as well and the measurement approaches
    pure steady-state device execution time. The fixed ~100 ms tunnel
    round-trip latency of the single await is amortized across the chain
    and its residual share is included (so this is still an upper bound
    on true per-exec HW time). Also measures single-shot wall latency
    for reference.

    Returns (best_amortized_seconds, dict with details).
    """
    import jax
    group, runners = _runners(neff_reps)
    dev_in = []
    for g, (run, _) in enumerate(runners):
        arrs = run.concat_inputs(in_maps[g * group:(g + 1) * group])
        dev_in.append([jax.device_put(a, run.sharding) for a in arrs])
    jax.block_until_ready(dev_in)

    # warm-up exec (first call compiles/loads the NEFF)
    outs = [run.jitted(*dev_in[g], *run.dev_zeros())
            for g, (run, _) in enumerate(runners)]
    jax.block_until_ready(outs)

    # single-shot latency (tunnel-dominated, for reference)
    single = []
    for _ in range(3):
        t0 = time.perf_counter()
        outs = [run.jitted(*dev_in[g], *outs[g])
                for g, (run, _) in enumerate(runners)]
        jax.block_until_ready(outs)
        single.append(time.perf_counter() - t0)

    # pipelined amortized timing
    amortized = []
    for _ in range(reps):
        t0 = time.perf_counter()
        for _ in range(pipeline):
            outs = [run.jitted(*dev_in[g], *outs[g])
                    for g, (run, _) in enumerate(runners)]
        jax.block_until_ready(outs)
        amortized.append((time.perf_counter() - t0) / (pipeline * neff_reps))
    return min(amortized), {
        "amortized": amortized,
        "single_shot": single,
        "pipeline": pipeline,
        "neff_reps": neff_reps,
    }


def prepare_in_maps(x, weight, bias):
    import ml_dtypes

    B, S, _ = x.shape
    xf = np.ascontiguousarray(x, dtype=np.float32).reshape(B * S, DIN)

    # scales (fp32 semantics, matching the jax reference)
    ax = np.float32(np.max(np.abs(xf)))
    sx = np.maximum(ax, np.float32(1e-8)) / np.float32(127.0)
    rx_val = np.float32(1.0) / sx
    wm = np.max(np.abs(weight), axis=1).astype(np.float32)
    sw = np.maximum(wm, np.float32(1e-8)) / np.float32(127.0)
    sc_v = (sx * sw).astype(np.float32)

    # host-side weight fake-quant (integer values, exact in bf16), transposed
    # to Din-major so the contraction dim lands on SBUF partitions
    wq = np.rint(np.clip(weight.astype(np.float32) / sw[:, None],
                         -127.0, 127.0)).astype(np.float32)
    wqT_v = np.ascontiguousarray(wq.T).astype(ml_dtypes.bfloat16)

    # [8, DIN, T] token shards, Din-major
    xsh = np.ascontiguousarray(
        xf.reshape(N_CORES, T, DIN).transpose(0, 2, 1))
    rx_col = np.full((P, 1), rx_val, np.float32)
    bias_v = np.ascontiguousarray(bias, dtype=np.float32)

    return [
        {"xT": xsh[c], "wqT": wqT_v, "sc": sc_v, "bi": bias_v, "rx": rx_col}
        for c in range(N_CORES)
    ]


def kernel(x: np.ndarray, weight: np.ndarray, bias: np.ndarray) -> np.ndarray:
    B, S, _ = x.shape
    in_maps = prepare_in_maps(x, weight, bias)
    group, runners = _runners()
    # jax dispatch is async: submit all groups, then block on results.
    pending = [
        run(in_maps[g * group:(g + 1) * group])
        for g, (run, _) in enumerate(runners)
    ]
    outs = []
    for (_, unpack), arrs in zip(runners, pending):
        outs.extend(r["y"] for r in unpack(arrs))
    y = np.concatenate(outs, axis=0)
    return y.reshape(B, S, DOUT).astype(np.float32)


# revision 23
# speedup vs baseline: 1.2345x; 1.2345x over previous
"""Fake-quantized linear layer (int8 symmetric fake-quant) on 8 TRN2 NeuronCores.

Reference computation:
    sx = max(|x|)/127            (per-tensor, scalar)
    sw[o] = max(|w[o,:]|)/127    (per-output-channel)
    qx = round(clip(x/sx, -127, 127));  qw = round(clip(w/sw, -127, 127))
    y = (qx*sx) @ (qw*sw).T + bias
      = (qx @ qw.T) * (sx*sw[o]) + bias    -- exact integer arithmetic

Device strategy (pure data-parallel over tokens, no collectives):
  - 16384 tokens sharded 2048/core; quantized weight + bias replicated.
  - Weights are quantized on the host (the standard int8-inference split:
    weights quantized offline, activations quantized on the fly); the
    integer-valued qw ships as bf16 (ints <= 127 are exact in bf16), already
    transposed to [Din, Dout] so the contraction dim lands on partitions.
  - Activations are quantized on device: x*(1/sx) then the fp32
    magic-constant round trick ((v + 12582912.0) - 12582912.0 ==
    round-half-even(v) for |v| < 2^22), emitted as bf16.
  - Matmul runs on the TensorEngine in bf16 with fp32 PSUM accumulation,
    then the output is scaled by sx*sw[o] and bias is added (VectorEngine),
    and written out f32.
  - SBUF (default variant "xdouble"): both 1024-token halves of quantized
    x are double-buffered resident ([128, 32, 1024] bf16 = 64 KiB/partition
    each) — no DRAM staging, no half-switch reload bubble, and in repeated
    execution the next invocation's quantization overlaps the current one's
    second-half matmuls. qw tiles ([128, 32, 512] bf16) are DMA'd directly
    (no on-device quant work), double-buffered; qw is read twice (once per
    token half). Older variants (n512_staged, n256_resident, dualres) are
    kept for reference; all measured slower.

Execution/timing notes (axon-tunneled PJRT):
  - Any client-side await (block_until_ready, device_put of even a few
    bytes) costs a fixed ~100 ms round trip through the tunnel. Back-to-back
    submitted executions pipeline on the terminal with ~1 ms marginal cost
    each (measured: 16 trivial execs complete in ~115 ms total; a chain of 8
    dependent execs in ~106 ms). So single-shot wall time is tunnel-latency
    dominated and says nothing about kernel speed.
  - bench() therefore measures sustained HW execution time: submit K
    donation-chained executions (exec i+1 donates exec i's output buffer, so
    they run strictly back-to-back on device), block once, divide by K.
  - All 8 cores run in a single shard_map dispatch.
"""

import os
import time

import numpy as np

import concourse.bacc as bacc
import concourse.mybir as mybir
import concourse.tile as tile
from concourse.bass_utils import run_bass_kernel_spmd  # noqa: F401 (debug path)

N_CORES = 8
P = 128
DIN = 4096
DOUT = 4096
T = 2048           # tokens per core
TH = 1024          # token half
KO = DIN // P      # 32 k-subtiles
NE = DOUT // 512   # 8 dout-eighths
MH = TH // P       # 8 m-subtiles per half
C_MAGIC = 12582912.0  # 2^23 + 2^22: fp32 round-to-nearest-even magic


def build(DIN=DIN, DOUT=DOUT, T=T, num_devices=N_CORES,
          psum_bufs=3, opool_bufs=3, xstage_bufs=2, variant="n512_staged",
          reps=1):
    """Build the kernel NEFF. reps>1 emits the full computation that many
    times back-to-back inside one NEFF (same inputs, same output buffer,
    each rep fully rewrites y) — used only for steady-state benchmarking;
    the result is identical to reps=1."""
    TH = T // 2
    KO = DIN // P
    NE = DOUT // 512
    MH = TH // P
    nc = bacc.Bacc("TRN2", target_bir_lowering=False, debug=False,
                   num_devices=num_devices)
    f32 = mybir.dt.float32
    bf16 = mybir.dt.bfloat16

    xT = nc.dram_tensor("xT", [DIN, T], f32, kind="ExternalInput")
    wqT = nc.dram_tensor("wqT", [DIN, DOUT], bf16, kind="ExternalInput")
    sc = nc.dram_tensor("sc", [DOUT], f32, kind="ExternalInput")     # sx*sw
    bi = nc.dram_tensor("bi", [DOUT], f32, kind="ExternalInput")     # bias
    rx = nc.dram_tensor("rx", [P, 1], f32, kind="ExternalInput")     # 1/sx
    y = nc.dram_tensor("y", [T, DOUT], f32, kind="ExternalOutput")

    if variant == "n256_resident":
        return _build_n256(nc, xT, wqT, sc, bi, rx, y,
                           DIN, DOUT, T, psum_bufs, opool_bufs)
    if variant == "dualres":
        return _build_dualres(nc, xT, wqT, sc, bi, rx, y,
                              DIN, DOUT, T, psum_bufs, opool_bufs, reps)
    if variant == "xdouble":
        return _build_xdouble(nc, xT, wqT, sc, bi, rx, y,
                              DIN, DOUT, T, psum_bufs, reps)

    with tile.TileContext(nc) as tc:
        with tc.tile_pool(name="xres", bufs=1) as xres, \
             tc.tile_pool(name="wq", bufs=2) as wqp, \
             tc.tile_pool(name="xstage", bufs=xstage_bufs) as xstage, \
             tc.tile_pool(name="xfstage", bufs=xstage_bufs) as xfstage, \
             tc.tile_pool(name="xq1stage", bufs=2) as xq1stage, \
             tc.tile_pool(name="rowbc", bufs=2) as rowbc, \
             tc.tile_pool(name="opool", bufs=opool_bufs) as opool, \
             tc.tile_pool(name="scal", bufs=1) as scal, \
             tc.tile_pool(name="dram", bufs=1, space="DRAM") as dram, \
             tc.tile_pool(name="psum", bufs=psum_bufs, space="PSUM") as psum:

            rxt = scal.tile([P, 1], f32)
            nc.sync.dma_start(rxt[:], rx.ap())

            def quant_x(c, ko, xq0, xq1_dram):
                xt = xstage.tile([P, TH], f32)
                nc.sync.dma_start(
                    xt[:], xT.ap()[ko * P:(ko + 1) * P, c * TH:(c + 1) * TH])
                xf = xfstage.tile([P, TH], f32)
                # xf = xt*(1/sx) + C  (scalar engine)
                nc.scalar.activation(xf[:], xt[:],
                                     mybir.ActivationFunctionType.Copy,
                                     bias=C_MAGIC, scale=rxt[:])
                # subtract C -> round-half-even(xt/sx), emit bf16
                if c == 0:
                    nc.vector.tensor_scalar(xq0[:, ko, :], xf[:], -C_MAGIC,
                                            None, mybir.AluOpType.add)
                else:
                    xq1 = xq1stage.tile([P, TH], bf16)
                    nc.vector.tensor_scalar(xq1[:], xf[:], -C_MAGIC,
                                            None, mybir.AluOpType.add)
                    nc.sync.dma_start(xq1_dram[ko], xq1[:])

            for _rep in range(reps):
                xq1_dram = dram.tile([KO, P, TH], bf16)

                # ---- Phase X: quantize x; half 0 -> resident SBUF,
                # half 1 -> DRAM staging (reloaded at half switch).
                xq0 = xres.tile([P, KO, TH], bf16, tag="xres")
                for ko in range(KO):
                    quant_x(0, ko, xq0, xq1_dram)
                for ko in range(KO):
                    quant_x(1, ko, xq0, xq1_dram)

                # ---- Main loop: token halves x dout eighths
                for h in range(2):
                    if h == 0:
                        xq = xq0
                    else:
                        xq = xres.tile([P, KO, TH], bf16, tag="xres")
                        nc.sync.dma_start(
                            xq[:], xq1_dram[:].rearrange("k p t -> p k t"))
                    for ne in range(NE):
                        ds = slice(ne * 512, (ne + 1) * 512)
                        scb = rowbc.tile([P, 512], f32, tag="scb")
                        nc.sync.dma_start(scb[:],
                                          sc.ap()[ds].partition_broadcast(P))
                        bib = rowbc.tile([P, 512], f32, tag="bib")
                        nc.sync.dma_start(bib[:],
                                          bi.ap()[ds].partition_broadcast(P))

                        wq = wqp.tile([P, KO, 512], bf16)
                        for ko in range(KO):
                            nc.sync.dma_start(
                                wq[:, ko, :], wqT.ap()[ko * P:(ko + 1) * P, ds])

                        for m in range(MH):
                            ps = psum.tile([P, 512], f32)
                            for k in range(KO):
                                nc.tensor.matmul(
                                    ps[:], xq[:, k, m * P:(m + 1) * P],
                                    wq[:, k, :],
                                    start=(k == 0), stop=(k == KO - 1))
                            ot = opool.tile([P, 512], f32)
                            nc.vector.tensor_tensor(ot[:], ps[:], scb[:],
                                                    mybir.AluOpType.mult)
                            nc.vector.tensor_tensor(ot[:], ot[:], bib[:],
                                                    mybir.AluOpType.add)
                            row = h * TH + m * P
                            nc.sync.dma_start(y.ap()[row:row + P, ds], ot[:])

    nc.compile()
    return nc


def _build_xdouble(nc, xT, wqT, sc, bi, rx, y, DIN, DOUT, T,
                   psum_bufs, reps):
    """Variant: both 1024-token halves of quantized x double-buffered in
    SBUF (2 x 64 KiB/partition) — no DRAM staging, no half-switch reload,
    and at rep boundaries the next rep's h0 quantization overlaps the
    current rep's h1 matmuls (the buffer frees as soon as h0's matmuls
    are done). Weight path identical to n512_staged (N=512, double-
    buffered qw blocks). Staging/broadcast pools shrunk to fit SBUF:
    quant in [128,512] chunks, single-buffered sc/bi broadcasts."""
    f32 = mybir.dt.float32
    bf16 = mybir.dt.bfloat16
    TH = T // 2
    KO = DIN // P
    NE = DOUT // 512
    MH = TH // P
    TQ = 512
    NCH = TH // TQ

    with tile.TileContext(nc) as tc:
        with tc.tile_pool(name="xres", bufs=2) as xres, \
             tc.tile_pool(name="wq", bufs=2) as wqp, \
             tc.tile_pool(name="xstage", bufs=2) as xstage, \
             tc.tile_pool(name="rowbc", bufs=1) as rowbc, \
             tc.tile_pool(name="opool", bufs=2) as opool, \
             tc.tile_pool(name="scal", bufs=1) as scal, \
             tc.tile_pool(name="psum", bufs=psum_bufs, space="PSUM") as psum:

            rxt = scal.tile([P, 1], f32)
            nc.sync.dma_start(rxt[:], rx.ap())

            for _rep in range(reps):
                xqs = []
                for h in range(2):
                    xq = xres.tile([P, KO, TH], bf16, tag="xres")
                    for c in range(NCH):
                        for ko in range(KO):
                            xt = xstage.tile([P, TQ], f32)
                            nc.sync.dma_start(
                                xt[:],
                                xT.ap()[ko * P:(ko + 1) * P,
                                        h * TH + c * TQ:h * TH + (c + 1) * TQ])
                            # in-place on the staging tile: xt = xt/sx + C
                            nc.scalar.activation(
                                xt[:], xt[:],
                                mybir.ActivationFunctionType.Copy,
                                bias=C_MAGIC, scale=rxt[:])
                            nc.vector.tensor_scalar(
                                xq[:, ko, c * TQ:(c + 1) * TQ], xt[:],
                                -C_MAGIC, None, mybir.AluOpType.add)
                    xqs.append(xq)

                for h in range(2):
                    xq = xqs[h]
                    for ne in range(NE):
                        ds = slice(ne * 512, (ne + 1) * 512)
                        scb = rowbc.tile([P, 512], f32, tag="scb")
                        nc.sync.dma_start(scb[:],
                                          sc.ap()[ds].partition_broadcast(P))
                        bib = rowbc.tile([P, 512], f32, tag="bib")
                        nc.sync.dma_start(bib[:],
                                          bi.ap()[ds].partition_broadcast(P))

                        wq = wqp.tile([P, KO, 512], bf16)
                        for ko in range(KO):
                            nc.sync.dma_start(
                                wq[:, ko, :],
                                wqT.ap()[ko * P:(ko + 1) * P, ds])

                        for m in range(MH):
                            ps = psum.tile([P, 512], f32)
                            for k in range(KO):
                                nc.tensor.matmul(
                                    ps[:], xq[:, k, m * P:(m + 1) * P],
                                    wq[:, k, :],
                                    start=(k == 0), stop=(k == KO - 1))
                            ot = opool.tile([P, 512], f32)
                            nc.vector.tensor_tensor(ot[:], ps[:], scb[:],
                                                    mybir.AluOpType.mult)
                            nc.vector.tensor_tensor(ot[:], ot[:], bib[:],
                                                    mybir.AluOpType.add)
                            row = h * TH + m * P
                            nc.sync.dma_start(y.ap()[row:row + P, ds], ot[:])

    nc.compile()
    return nc


def _build_dualres(nc, xT, wqT, sc, bi, rx, y, DIN, DOUT, T,
                   psum_bufs, opool_bufs, reps):
    """Variant: full quantized-x residency ([128, 32, 2048] bf16 =
    128 KiB/partition) at N=512, no DRAM staging. Weight blocks are
    double-buffered at half-K granularity ([128, 16, 512] bf16 = 16 KiB,
    3 bufs) so everything fits in SBUF. x is quantized in token-major
    order so the first matmul group is gated on only 1/4 of the x DMA."""
    f32 = mybir.dt.float32
    bf16 = mybir.dt.bfloat16
    KO = DIN // P
    KO2 = KO // 2
    NE = DOUT // 512
    MT = T // P
    TQ = 512
    NQ = T // TQ

    with tile.TileContext(nc) as tc:
        with tc.tile_pool(name="xres", bufs=1) as xres, \
             tc.tile_pool(name="wq", bufs=3) as wqp, \
             tc.tile_pool(name="xstage", bufs=2) as xstage, \
             tc.tile_pool(name="xfstage", bufs=2) as xfstage, \
             tc.tile_pool(name="rowbc", bufs=2) as rowbc, \
             tc.tile_pool(name="opool", bufs=opool_bufs) as opool, \
             tc.tile_pool(name="scal", bufs=1) as scal, \
             tc.tile_pool(name="psum", bufs=psum_bufs, space="PSUM") as psum:

            rxt = scal.tile([P, 1], f32)
            nc.sync.dma_start(rxt[:], rx.ap())

            for _rep in range(reps):
                xq = xres.tile([P, KO, T], bf16, tag="xres")
                for c in range(NQ):
                    for ko in range(KO):
                        xt = xstage.tile([P, TQ], f32)
                        nc.sync.dma_start(
                            xt[:],
                            xT.ap()[ko * P:(ko + 1) * P,
                                    c * TQ:(c + 1) * TQ])
                        xf = xfstage.tile([P, TQ], f32)
                        nc.scalar.activation(
                            xf[:], xt[:],
                            mybir.ActivationFunctionType.Copy,
                            bias=C_MAGIC, scale=rxt[:])
                        nc.vector.tensor_scalar(
                            xq[:, ko, c * TQ:(c + 1) * TQ], xf[:], -C_MAGIC,
                            None, mybir.AluOpType.add)

                for ne in range(NE):
                    ds = slice(ne * 512, (ne + 1) * 512)
                    scb = rowbc.tile([P, 512], f32, tag="scb")
                    nc.sync.dma_start(scb[:],
                                      sc.ap()[ds].partition_broadcast(P))
                    bib = rowbc.tile([P, 512], f32, tag="bib")
                    nc.sync.dma_start(bib[:],
                                      bi.ap()[ds].partition_broadcast(P))

                    wk = []
                    for kh in range(2):
                        wt = wqp.tile([P, KO2, 512], bf16)
                        for k in range(KO2):
                            ko = kh * KO2 + k
                            nc.sync.dma_start(
                                wt[:, k, :],
                                wqT.ap()[ko * P:(ko + 1) * P, ds])
                        wk.append(wt)

                    for m in range(MT):
                        ps = psum.tile([P, 512], f32)
                        for kh in range(2):
                            for k in range(KO2):
                                ko = kh * KO2 + k
                                nc.tensor.matmul(
                                    ps[:], xq[:, ko, m * P:(m + 1) * P],
                                    wk[kh][:, k, :],
                                    start=(ko == 0), stop=(ko == KO - 1))
                        ot = opool.tile([P, 512], f32)
                        nc.vector.tensor_tensor(ot[:], ps[:], scb[:],
                                                mybir.AluOpType.mult)
                        nc.vector.tensor_tensor(ot[:], ot[:], bib[:],
                                                mybir.AluOpType.add)
                        nc.sync.dma_start(y.ap()[m * P:(m + 1) * P, ds], ot[:])

    nc.compile()
    return nc


def _build_n256(nc, xT, wqT, sc, bi, rx, y, DIN, DOUT, T,
                psum_bufs, opool_bufs):
    """Variant: full quantized-x residency ([128, 32, 2048] bf16 =
    128 KiB/partition), no DRAM staging and no half-switch reload bubble;
    weight blocks narrowed to 256 outs so double-buffered qw still fits."""
    f32 = mybir.dt.float32
    bf16 = mybir.dt.bfloat16
    TH = T // 2
    KO = DIN // P
    NB = 256
    NEB = DOUT // NB
    MT = T // P

    with tile.TileContext(nc) as tc:
        with tc.tile_pool(name="xres", bufs=1) as xres, \
             tc.tile_pool(name="wq", bufs=2) as wqp, \
             tc.tile_pool(name="xstage", bufs=2) as xstage, \
             tc.tile_pool(name="xfstage", bufs=2) as xfstage, \
             tc.tile_pool(name="rowbc", bufs=2) as rowbc, \
             tc.tile_pool(name="opool", bufs=opool_bufs) as opool, \
             tc.tile_pool(name="scal", bufs=1) as scal, \
             tc.tile_pool(name="psum", bufs=psum_bufs, space="PSUM") as psum:

            rxt = scal.tile([P, 1], f32)
            nc.sync.dma_start(rxt[:], rx.ap())

            xq = xres.tile([P, KO, T], bf16)
            for c in range(2):
                for ko in range(KO):
                    xt = xstage.tile([P, TH], f32)
                    nc.sync.dma_start(
                        xt[:],
                        xT.ap()[ko * P:(ko + 1) * P, c * TH:(c + 1) * TH])
                    xf = xfstage.tile([P, TH], f32)
                    nc.scalar.activation(xf[:], xt[:],
                                         mybir.ActivationFunctionType.Copy,
                                         bias=C_MAGIC, scale=rxt[:])
                    nc.vector.tensor_scalar(
                        xq[:, ko, c * TH:(c + 1) * TH], xf[:], -C_MAGIC,
                        None, mybir.AluOpType.add)

            for ne in range(NEB):
                ds = slice(ne * NB, (ne + 1) * NB)
                scb = rowbc.tile([P, NB], f32, tag="scb")
                nc.sync.dma_start(scb[:], sc.ap()[ds].partition_broadcast(P))
                bib = rowbc.tile([P, NB], f32, tag="bib")
                nc.sync.dma_start(bib[:], bi.ap()[ds].partition_broadcast(P))

                wq = wqp.tile([P, KO, NB], bf16)
                for ko in range(KO):
                    nc.sync.dma_start(
                        wq[:, ko, :], wqT.ap()[ko * P:(ko + 1) * P, ds])

                for m in range(MT):
                    ps = psum.tile([P, NB], f32)
                    for k in range(KO):
                        nc.tensor.matmul(
                            ps[:], xq[:, k, m * P:(m + 1) * P], wq[:, k, :],
                            start=(k == 0), stop=(k == KO - 1))
                    ot = opool.tile([P, NB], f32)
                    nc.vector.tensor_tensor(ot[:], ps[:], scb[:],
                                            mybir.AluOpType.mult)
                    nc.vector.tensor_tensor(ot[:], ot[:], bib[:],
                                            mybir.AluOpType.add)
                    nc.sync.dma_start(y.ap()[m * P:(m + 1) * P, ds], ot[:])

    nc.compile()
    return nc


_NC_CACHE = {}


def _get_nc(reps=1, variant=None):
    if variant is None:
        variant = os.environ.get("KERNEL_VARIANT", "xdouble")
    key = ("nc", reps, variant)
    if key not in _NC_CACHE:
        _NC_CACHE[key] = build(variant=variant, reps=reps)
    return _NC_CACHE[key]


def _get_runner(dev_lo, dev_hi, reps=1, variant=None):
    """Compiled shard_map runner for jax devices [dev_lo, dev_hi).

    Mirrors concourse.bass2jax.run_bass_via_pjrt's multi-core path, but
    caches the jitted executable and exposes helpers for device-resident
    pipelined benching (donation-chained repeat executions).
    """
    if variant is None:
        variant = os.environ.get("KERNEL_VARIANT", "xdouble")
    key = (dev_lo, dev_hi, reps, variant)
    if key in _NC_CACHE:
        return _NC_CACHE[key]

    import jax
    import jax.numpy as jnp
    from jax.sharding import Mesh, PartitionSpec
    from jax.experimental.shard_map import shard_map
    from concourse import bass2jax, mybir as _mybir

    nc = _get_nc(reps, variant)
    bass2jax.install_neuronx_cc_hook()

    partition_name = (nc.partition_id_tensor.name
                      if nc.partition_id_tensor else None)
    in_names, out_names, out_avals, zero_outs = [], [], [], []
    for alloc in nc.m.functions[0].allocations:
        if not isinstance(alloc, _mybir.MemoryLocationSet):
            continue
        name = alloc.memorylocations[0].name
        if alloc.kind == "ExternalInput":
            if name != partition_name:
                in_names.append(name)
        elif alloc.kind == "ExternalOutput":
            shape = tuple(alloc.tensor_shape)
            dtype = _mybir.dt.np(alloc.dtype)
            out_names.append(name)
            out_avals.append(jax.core.ShapedArray(shape, dtype))
            zero_outs.append(np.zeros(shape, dtype))
    n_params = len(in_names)
    n_outs = len(out_avals)
    all_names = in_names + out_names
    if partition_name is not None:
        all_names = all_names + [partition_name]
    donate = tuple(range(n_params, n_params + n_outs))
    n_cores = dev_hi - dev_lo

    def _body(*args):
        operands = list(args)
        if partition_name is not None:
            operands.append(bass2jax.partition_id_tensor())
        outs = bass2jax._bass_exec_p.bind(
            *operands,
            out_avals=tuple(out_avals),
            in_names=tuple(all_names),
            out_names=tuple(out_names),
            lowering_input_output_aliases=(),
            sim_require_finite=True,
            sim_require_nnan=True,
            nc=nc,
        )
        return tuple(outs)

    devices = jax.devices()[dev_lo:dev_hi]
    mesh = Mesh(np.asarray(devices), ("core",))
    in_specs = (PartitionSpec("core"),) * (n_params + n_outs)
    out_specs = (PartitionSpec("core"),) * n_outs
    jitted = jax.jit(
        shard_map(_body, mesh=mesh, in_specs=in_specs, out_specs=out_specs,
                  check_rep=False),
        donate_argnums=donate, keep_unused=True)

    sharding = jax.sharding.NamedSharding(mesh, PartitionSpec("core"))

    def concat_inputs(in_maps):
        assert len(in_maps) == n_cores
        return [
            np.concatenate([np.asarray(m[name]) for m in in_maps], axis=0)
            for name in in_names
        ]

    # Donated output buffers are created on device (jnp.zeros under jit) --
    # the kernel writes every output element, so contents don't matter, but
    # this avoids shipping 100s of MB of host zeros through the tunnel.
    zshapes = [((n_cores * z.shape[0],) + z.shape[1:], z.dtype)
               for z in zero_outs]
    dev_zeros = jax.jit(
        lambda: tuple(jnp.zeros(s, d) for s, d in zshapes),
        out_shardings=tuple(sharding for _ in zshapes))

    def run(in_maps):
        dev_in = [jax.device_put(a, sharding) for a in concat_inputs(in_maps)]
        return jitted(*dev_in, *dev_zeros())

    run.jitted = jitted
    run.concat_inputs = concat_inputs
    run.dev_zeros = dev_zeros
    run.sharding = sharding

    def unpack(out_arrs):
        return [
            {name: np.asarray(out_arrs[i]).reshape(
                n_cores, *out_avals[i].shape)[c]
             for i, name in enumerate(out_names)}
            for c in range(n_cores)
        ]

    _NC_CACHE[key] = (run, unpack)
    return _NC_CACHE[key]


def _runners(neff_reps=1, variant=None):
    group = int(os.environ.get("KERNEL_CORE_GROUP", "8"))
    return group, [_get_runner(g0, g0 + group, reps=neff_reps,
                               variant=variant)
                   for g0 in range(0, N_CORES, group)]


def bench(in_maps, reps=5, pipeline=64, neff_reps=1, variant=None):
    """Measure sustained per-execution HW time.

    Inputs are device_put once (outside any timer). Each rep submits
    `pipeline` donation-chained executions (exec i+1 consumes exec i's
    output buffer, so they run strictly back-to-back on the device) and
    blocks once; rep time = total / (pipeline * neff_reps). With
    neff_reps>1 each execution's NEFF contains that many back-to-back
    repetitions of the full kernel computation, so per-kernel runtime
    dispatch overhead is amortized as well and the measurement approaches
    pure steady-state device execution time. The fixed ~100 ms tunnel
    round-trip latency of the single await is amortized across the chain
    and its residual share is included (so this is still an upper bound
    on true per-exec HW time). Also measures single-shot wall latency
    for reference.

    Returns (best_amortized_seconds, dict with details).
    """
    import jax
    group, runners = _runners(neff_reps, variant)
    dev_in = []
    for g, (run, _) in enumerate(runners):
        arrs = run.concat_inputs(in_maps[g * group:(g + 1) * group])
        dev_in.append([jax.device_put(a, run.sharding) for a in arrs])
    jax.block_until_ready(dev_in)

    # warm-up exec (first call compiles/loads the NEFF)
    outs = [run.jitted(*dev_in[g], *run.dev_zeros())
            for g, (run, _) in enumerate(runners)]
    jax.block_until_ready(outs)

    # single-shot latency (tunnel-dominated, for reference)
    single = []
    for _ in range(3):
        t0 = time.perf_counter()
        outs = [run.jitted(*dev_in[g], *outs[g])
                for g, (run, _) in enumerate(runners)]
        jax.block_until_ready(outs)
        single.append(time.perf_counter() - t0)

    # pipelined amortized timing
    amortized = []
    for _ in range(reps):
        t0 = time.perf_counter()
        for _ in range(pipeline):
            outs = [run.jitted(*dev_in[g], *outs[g])
                    for g, (run, _) in enumerate(runners)]
        jax.block_until_ready(outs)
        amortized.append((time.perf_counter() - t0) / (pipeline * neff_reps))
    return min(amortized), {
        "amortized": amortized,
        "single_shot": single,
        "pipeline": pipeline,
        "neff_reps": neff_reps,
    }


def prepare_in_maps(x, weight, bias):
    import ml_dtypes

    B, S, _ = x.shape
    xf = np.ascontiguousarray(x, dtype=np.float32).reshape(B * S, DIN)

    # scales (fp32 semantics, matching the jax reference)
    ax = np.float32(np.max(np.abs(xf)))
    sx = np.maximum(ax, np.float32(1e-8)) / np.float32(127.0)
    rx_val = np.float32(1.0) / sx
    wm = np.max(np.abs(weight), axis=1).astype(np.float32)
    sw = np.maximum(wm, np.float32(1e-8)) / np.float32(127.0)
    sc_v = (sx * sw).astype(np.float32)

    # host-side weight fake-quant (integer values, exact in bf16), transposed
    # to Din-major so the contraction dim lands on SBUF partitions
    wq = np.rint(np.clip(weight.astype(np.float32) / sw[:, None],
                         -127.0, 127.0)).astype(np.float32)
    wqT_v = np.ascontiguousarray(wq.T).astype(ml_dtypes.bfloat16)

    # [8, DIN, T] token shards, Din-major
    xsh = np.ascontiguousarray(
        xf.reshape(N_CORES, T, DIN).transpose(0, 2, 1))
    rx_col = np.full((P, 1), rx_val, np.float32)
    bias_v = np.ascontiguousarray(bias, dtype=np.float32)

    return [
        {"xT": xsh[c], "wqT": wqT_v, "sc": sc_v, "bi": bias_v, "rx": rx_col}
        for c in range(N_CORES)
    ]


def kernel(x: np.ndarray, weight: np.ndarray, bias: np.ndarray) -> np.ndarray:
    B, S, _ = x.shape
    in_maps = prepare_in_maps(x, weight, bias)
    group, runners = _runners()
    # jax dispatch is async: submit all groups, then block on results.
    pending = [
        run(in_maps[g * group:(g + 1) * group])
        for g, (run, _) in enumerate(runners)
    ]
    outs = []
    for (_, unpack), arrs in zip(runners, pending):
        outs.extend(r["y"] for r in unpack(arrs))
    y = np.concatenate(outs, axis=0)
    return y.reshape(B, S, DOUT).astype(np.float32)
